# revision 38
# baseline (speedup 1.0000x reference)
"""Trainium2 Bass kernel for nn_AttentionRNN (embedding + masked GRU + MLP head + softmax).

Strategy (pure data parallelism over 8 NeuronCores, 2048 examples/core):

Layout: everything transposed — state h kept as hT [H=128 partitions, examples
on free dim], so the GRU recurrence is closed under the layout (no per-step
transposes). Per time step t, per 512-example group (psum tile [128, 1536] f32):

  psum[:, 0:512]    = U_z.T @ hT + M_z.T @ xghT + 1s.T @ notm_t   (z preact)
  psum[:, 512:1024] = U_r.T @ hT + M_r.T @ xghT                   (r preact)
  psum[:, 1024:1536]= U_h.T @ hT                                  (rec_h)
  z|r  = sigmoid(psum[:, 0:1024])                 (one ACT call, reads PSUM)
  t1   = (rec_h + b1_h) * r                       (fused DVE scalar_tensor_tensor)
  t2   = t1 + xgh
  hh   = tanh(t2)
  h'   = z*(h - hh) + hh                          (3 DVE tensor_tensor ops)

The only gather is xghT: dma_gather (transpose mode) from a host-precomputed
fp16 table gtab[V, 128] = emb @ W[:, 256:384] + b0_h  (256B rows).  The z/r
input projections are reconstructed algebraically instead of gathered:
  x = (xgh - b0_h) @ pinv(W_h)  (exact: xgh lies in W_h's 32-dim row space)
  xg_zr = x @ W_zr = xgh @ M + c,  M = pinv(W_h) @ W_zr  (host-precomputed)
so no second gather is needed.  dma_gather is HW-limited to ~896 idxs/call
(1024+ kills the Pool engine), so each step gathers in (896, 896, 256) splits.

Mask (token==0 freezes state): notmT[t, i] = 100 if token==0 else 0, shipped
from host; a K=1 matmul adds it to the z preactivation => z = sigmoid(.+100) = 1
exactly => h' = h.  Biases: b0_h folded into gtab; b1_h via the STT scalar;
b0/b1_zr (+ the -b0_h@M correction) via K=1 matmuls only when nonzero.

Head: dT = swish(W1.T @ hT + b1); logits per 128-example tile with examples on
partitions (lhsT = dT slice); softmax along free dim (C=3).
"""

import numpy as np
from contextlib import ExitStack

import concourse.mybir as mybir
import concourse.tile as tile
from concourse import bacc
from concourse.bass_utils import run_bass_kernel_spmd

B, T, E, H, V, D, C = 16384, 128, 32, 128, 30001, 128, 3
NCORES = 8
BC = B // NCORES
BIGM = 100.0
NIDX = 896             # max idxs per dma_gather call (HW-probed ucode limit:
                       # 896 works, 1024+ crashes the Pool engine)
USE_SPLIT = True       # split zr/g psum tiles vs one 3-bank tile
G_BUFS = 6
H_BUFS = 4
Z_BUFS = 4
TMP_BUFS = 3
NM_CH = 2
PW = 512
SIG_SPLIT = True
TANH_MERGE = True
HEAD_SHARE_PS = False
R_FIRST = True
SKIP_GATHER = False
GSPLIT_OVERRIDE = None
SGW = 512              # psum group width (cols per z/r/g psum tile)
PS_BUFS = 2            # psum ring depth for pz/pr/pg pools

F16 = mybir.dt.float16
F32 = mybir.dt.float32
I16 = mybir.dt.int16
AF = mybir.ActivationFunctionType
OP = mybir.AluOpType
AX = mybir.AxisListType


def build_nc(bc=BC, nt=T, with_czr=False):
    """Build + compile the per-core Bass program. bc = examples per core."""
    assert bc % 512 == 0
    ng = bc // 512            # 512-example groups per step
    pw = min(PW, bc)          # width of the wide DVE ops
    npairs = bc // pw
    gperp = pw // 512
    gsplit = []
    off = 0
    while off < bc:
        n = min(NIDX, bc - off)
        gsplit.append((off, n))
        off += n
    if GSPLIT_OVERRIDE:
        gsplit = GSPLIT_OVERRIDE

    nc = bacc.Bacc("TRN2", target_bir_lowering=False, debug=False,
                   num_swdge_queues=4)
    gtab = nc.dram_tensor("gtab", [V, 128], F16, kind="ExternalInput").ap()
    idxw = nc.dram_tensor("idxw", [128, nt * bc // 16], I16, kind="ExternalInput").ap()
    uzrh = nc.dram_tensor("uzrh", [128, 384], F16, kind="ExternalInput").ap()
    mzr = nc.dram_tensor("mzr", [128, 256], F16, kind="ExternalInput").ap()
    b1h = nc.dram_tensor("b1h", [128, 1], F32, kind="ExternalInput").ap()
    w1 = nc.dram_tensor("w1", [128, 128], F16, kind="ExternalInput").ap()
    b1c = nc.dram_tensor("b1c", [128, 1], F32, kind="ExternalInput").ap()
    wout = nc.dram_tensor("wout", [128, C], F16, kind="ExternalInput").ap()
    boutw = nc.dram_tensor("boutw", [1, C], F16, kind="ExternalInput").ap()
    if with_czr:
        czr = nc.dram_tensor("czr", [1, 256], F16, kind="ExternalInput").ap()
    outp = nc.dram_tensor("outp", [128, (bc // 128) * C], F32, kind="ExternalOutput").ap()

    with tile.TileContext(nc) as tc, ExitStack() as ctx:
        wp = ctx.enter_context(tc.tile_pool(name="w", bufs=1))
        ip = ctx.enter_context(tc.tile_pool(name="idx", bufs=1))
        gp = ctx.enter_context(tc.tile_pool(name="g", bufs=G_BUFS))
        hp = ctx.enter_context(tc.tile_pool(name="h", bufs=H_BUFS))
        zp = ctx.enter_context(tc.tile_pool(name="zr", bufs=Z_BUFS))
        tp = ctx.enter_context(tc.tile_pool(name="tmp", bufs=TMP_BUFS))
        ppz = ctx.enter_context(tc.tile_pool(name="ppz", bufs=PS_BUFS, space="PSUM"))
        pprg = ctx.enter_context(tc.tile_pool(name="pprg", bufs=PS_BUFS, space="PSUM"))
        ppg = ctx.enter_context(tc.tile_pool(name="ppg", bufs=PS_BUFS, space="PSUM"))
        ph = ctx.enter_context(tc.tile_pool(name="ph", bufs=2, space="PSUM"))
        hd = ctx.enter_context(tc.tile_pool(name="hd", bufs=2))

        u_sb = wp.tile([128, 384], F16, tag="u")
        nc.sync.dma_start(u_sb[:], uzrh)
        m_sb = wp.tile([128, 256], F16, tag="mzr")
        nc.sync.dma_start(m_sb[:], mzr)
        b1h_sb = wp.tile([128, 1], F32, tag="b1h")
        nc.sync.dma_start(b1h_sb[:], b1h)
        w1_sb = wp.tile([128, 128], F16, tag="w1")
        nc.sync.dma_start(w1_sb[:], w1)
        b1c_sb = wp.tile([128, 1], F32, tag="b1c")
        nc.sync.dma_start(b1c_sb[:], b1c)
        wout_sb = wp.tile([128, C], F16, tag="wo")
        nc.sync.dma_start(wout_sb[:], wout)
        bout_sb = wp.tile([1, C], F16, tag="bo")
        nc.sync.dma_start(bout_sb[:], boutw)
        ones_sb = wp.tile([1, 128], F16, tag="ones")
        nc.vector.memset(ones_sb[:], 1.0)
        # Pin the ACT table set that contains BOTH Sigmoid and Tanh so the
        # auto-placement pass doesn't ping-pong table loads every step
        # (~1.3us per load on the ACT critical path).
        from concourse.hw_specs import get_activation_tables
        _tabs = get_activation_tables(nc.m.arch)
        _setid = next(i for i, (nm2, fs) in enumerate(_tabs.items())
                      if AF.Sigmoid in fs and AF.Tanh in fs)
        nc.scalar.add_instruction(mybir.InstLoadActFuncSet(
            name=nc.get_next_instruction_name(), ins=[], outs=[],
            act_func_set_id=_setid))
        if with_czr:
            czr_sb = wp.tile([1, 256], F16, tag="czr")
            nc.sync.dma_start(czr_sb[:], czr)
            onesbc_sb = wp.tile([1, bc], F16, tag="onesbc")
            nc.vector.memset(onesbc_sb[:], 1.0)
        idx_sb = ip.tile([128, nt * bc // 16], I16, tag="idx")
        nc.sync.dma_start(idx_sb[:], idxw)

        h = hp.tile([128, bc], F16, tag="h")
        nc.vector.memset(h[:], 0.0)

        _gq = [0]              # global gather-call counter for queue RR
        SG = min(SGW, bc)      # psum group width
        nsg = bc // SG
        BW = min(1024, bc)     # blend width (pairs groups when SG=512)
        gpb = BW // SG         # groups per blend

        def do_gather(t):
            g = gp.tile([128, 1, bc], F16, tag="g")
            for off, n in gsplit:
                nc.gpsimd.dma_gather(
                    g[:, :, off:off + n], gtab,
                    idx_sb[:, (t * bc + off) // 16:(t * bc + off + n) // 16],
                    n, n, 128, transpose=True,
                    queue_num=_gq[0] % 4,
                )
                _gq[0] += 1
            return g

        for t in range(nt):
            g = do_gather(t)
            xgh = g[:, 0, :]
            # zr layout: z in [0:bc], r in [bc:2bc] (contiguous halves so the
            # wide blend reads stay in DVE 2x mode)
            zrt = zp.tile([128, 2 * bc], F16, tag="zr")
            t1 = tp.tile([128, bc], F16, tag="t1")
            t2 = tp.tile([128, bc], F16, tag="t2")
            hh = tp.tile([128, bc], F16, tag="hh")
            dd = tp.tile([128, bc], F16, tag="dd")
            m1 = tp.tile([128, bc], F16, tag="m1")
            hnew = hp.tile([128, bc], F16, tag="h")

            pzs = [None] * nsg
            prs = [None] * nsg
            pgs = [None] * nsg
            nh = SG // 512           # matmul output <= 1 psum bank (512 f32)

            def emit_mms(sg):
                exs_base = sg * SG
                pz = ppz.tile([128, SG], F32, tag="pz")
                pr = pprg.tile([128, SG], F32, tag="prg")
                pG = ppg.tile([128, SG], F32, tag="pg")
                pzs[sg], prs[sg], pgs[sg] = pz, pr, pG

                def mm(ps_t, w, rhs, start, stop):
                    for q in range(nh):
                        qs = slice(q * 512, (q + 1) * 512)
                        rs = slice(exs_base + q * 512, exs_base + (q + 1) * 512)
                        nc.tensor.matmul(ps_t[:, qs], w, rhs[:, rs],
                                         start=start, stop=stop)

                # gather-only prefills (r first: its psum frees earlier)
                mm(pr, m_sb[:, 128:256], xgh, True, False)
                mm(pz, m_sb[:, 0:128], xgh, True, False)
                if with_czr:
                    mm(pr, czr_sb[:, 128:256], onesbc_sb, False, False)
                    mm(pz, czr_sb[:, 0:128], onesbc_sb, False, False)
                # h-dependent accumulations
                mm(pr, u_sb[:, 128:256], h, False, True)
                mm(pG, u_sb[:, 256:384], h, True, True)
                mm(pz, u_sb[:, 0:128], h, False, True)

            def emit_sig(sg):
                zsl = slice(sg * SG, (sg + 1) * SG)
                rsl = slice(bc + sg * SG, bc + (sg + 1) * SG)
                nc.scalar.activation(zrt[:, rsl], prs[sg][:], AF.Sigmoid)
                nc.scalar.activation(zrt[:, zsl], pzs[sg][:], AF.Sigmoid)

            def emit_stt(sg):
                exs = slice(sg * SG, (sg + 1) * SG)
                rsl = slice(bc + sg * SG, bc + (sg + 1) * SG)
                nc.vector.scalar_tensor_tensor(
                    t1[:, exs], pgs[sg][:], b1h_sb[:], zrt[:, rsl],
                    OP.add, OP.mult)

            def emit_blend(pi):
                exs = slice(pi * BW, (pi + 1) * BW)
                # blend: h' = z*(h - hh) + hh
                nc.vector.tensor_add(t2[:, exs], t1[:, exs], xgh[:, exs])
                nc.scalar.activation(hh[:, exs], t2[:, exs], AF.Tanh)
                nc.vector.tensor_sub(dd[:, exs], h[:, exs], hh[:, exs])
                nc.vector.tensor_mul(m1[:, exs], zrt[:, exs], dd[:, exs])
                nc.vector.tensor_add(hnew[:, exs], m1[:, exs], hh[:, exs])

            for sg in range(nsg):
                emit_mms(sg)
                emit_sig(sg)
                emit_stt(sg)
                if sg % gpb == gpb - 1:
                    emit_blend(sg // gpb)
            h = hnew

        out_sb = hd.tile([128, (bc // 128) * C], F32, tag="out")
        et_all = hd.tile([128, (bc // 128) * C], F32, tag="eta")
        ss_all = hd.tile([128, (bc // 128)], F32, tag="ssa")
        for hg in range(bc // 512):
            psd_t = ph.tile([128, 512], F32, tag="hps")
            psd = psd_t[:]
            nc.tensor.matmul(psd, w1_sb[:], h[:, hg * 512:(hg + 1) * 512], start=True, stop=True)
            sg = hd.tile([128, 512], F16, tag="sg")
            nc.scalar.activation(sg[:], psd, AF.Sigmoid, bias=b1c_sb[:])
            dt = hd.tile([128, 512], F16, tag="dt")
            # swish(d) = d * sigmoid(d), d = psd + b1
            nc.vector.scalar_tensor_tensor(dt[:], psd, b1c_sb[:], sg[:], OP.add, OP.mult)
            for sub in range(4):
                psl_t = ph.tile([128, C], F32, tag="hps")
                psl = psl_t[:]
                nc.tensor.matmul(psl, dt[:, sub * 128:(sub + 1) * 128], wout_sb[:], start=True, stop=False)
                nc.tensor.matmul(psl, ones_sb[:], bout_sb[:], start=False, stop=True)
                i = hg * 4 + sub
                nc.scalar.activation(et_all[:, i * C:(i + 1) * C], psl, AF.Exp,
                                     accum_out=ss_all[:, i:i + 1])
        rc_all = hd.tile([128, (bc // 128)], F32, tag="rc")
        nc.vector.reciprocal(rc_all[:], ss_all[:])
        for i in range(bc // 128):
            nc.vector.tensor_scalar_mul(out_sb[:, i * C:(i + 1) * C],
                                        et_all[:, i * C:(i + 1) * C], rc_all[:, i:i + 1])
        nc.sync.dma_start(outp, out_sb[:])

    nc.compile()
    return nc


def prep_tables(emb, W, U, b, W1, b1, Wout, bout):
    """Host-side weight preprocessing -> (shared input dict, with_czr flag).

    Mask trick: real-token rows of gtab lie in L = span(Wh rows, b0_h)
    (dim<=33).  Pick u ⊥ L; add alpha*u to token-0's row and u⊗[BIGM/alpha
    (z-slots), 0 (r-slots)] to M.  For real tokens u.T@xgh = 0 so nothing
    changes; for token 0 the z preact gets +BIGM => z = 1 => h frozen.
    """
    f16 = np.float16
    emb = np.asarray(emb, np.float64)
    W = np.asarray(W, np.float64)
    b = np.asarray(b, np.float64)
    Wh = W[:, 256:384]
    gtab64 = emb @ Wh + b[0, 256:384]                       # [V, 128]
    Minv = np.linalg.pinv(Wh)                               # [128, 32]
    M = Minv @ W[:, 0:256]                                  # [128, 256]
    # u orthogonal to L = span(Wh.T cols, b0_h)
    Lb = np.concatenate([Wh.T, b[0, 256:384].reshape(128, 1)], axis=1)
    Q, _ = np.linalg.qr(Lb)                                 # [128, 33]
    rng = np.random.default_rng(12345)
    gvec = rng.standard_normal(128)
    u = gvec - Q @ (Q.T @ gvec)
    u /= np.linalg.norm(u)
    ALPHA = 16.0
    gtab64[0] += ALPHA * u
    madd = np.concatenate([np.full(128, BIGM / ALPHA), np.zeros(128)])
    M = M + np.outer(u, madd)
    gtab = gtab64.astype(f16)
    M = M.astype(f16)
    # xg_zr = (xgh - b0_h) @ M + b0_zr  (+ b1_zr folded with it)
    c = (-b[0, 256:384] @ Minv @ W[:, 0:256] + b[0, 0:256] + b[1, 0:256])
    with_czr = bool(np.any(np.abs(c) > 1e-12))
    shared = {
        "gtab": gtab,
        "uzrh": np.asarray(U, np.float32).astype(f16),
        "mzr": M,  # already f16
        "b1h": np.asarray(b[1, 256:384], np.float32).reshape(128, 1).copy(),
        "w1": np.asarray(W1, np.float32).astype(f16),
        "b1c": np.asarray(b1, np.float32).reshape(128, 1).copy(),
        "wout": np.asarray(Wout, np.float32).astype(f16),
        "boutw": np.asarray(bout, np.float32).reshape(1, C).astype(f16),
    }
    if with_czr:
        shared["czr"] = c.reshape(1, 256).astype(f16)
    return shared, with_czr


def prep_idx(tokens_core, nt):
    """tokens_core [bc, nt] int -> wrapped idx tensor [128, nt*bc/16] int16."""
    bc = tokens_core.shape[0]
    tk = np.ascontiguousarray(tokens_core.astype(np.int16))
    w = tk.T.reshape(nt, bc // 16, 16).transpose(0, 2, 1)   # [t, r, c16]
    w = np.tile(w, (1, 8, 1))
    return np.ascontiguousarray(w.transpose(1, 0, 2).reshape(128, nt * bc // 16))


def assemble_out(res_core, bc=BC):
    """[128, (bc/128)*3] f32 device output -> [bc, 3]."""
    return np.ascontiguousarray(
        res_core.reshape(128, bc // 128, C).transpose(1, 0, 2).reshape(bc, C)
    )


_NC_CACHE = {}


def kernel(tokens, emb, W, U, b, W1, b1, Wout, bout):
    tokens = np.asarray(tokens)
    shared, with_czr = prep_tables(emb, W, U, b, W1, b1, Wout, bout)
    key = (BC, T, with_czr)
    if key not in _NC_CACHE:
        _NC_CACHE[key] = build_nc(BC, T, with_czr)
    nc = _NC_CACHE[key]
    in_maps = []
    for c in range(NCORES):
        m = dict(shared)
        tc = tokens[c * BC:(c + 1) * BC]
        m["idxw"] = prep_idx(tc, T)
        in_maps.append(m)
    res = run_bass_kernel_spmd(nc, in_maps, core_ids=list(range(NCORES)))
    out = np.concatenate([assemble_out(res.results[c]["outp"], BC) for c in range(NCORES)], axis=0)
    return out.astype(np.float32)



# revision 39
# speedup vs baseline: 1.1403x; 1.1403x over previous
"""Trainium2 Bass kernel for nn_AttentionRNN (embedding + masked GRU + MLP head + softmax).

Strategy (pure data parallelism over 8 NeuronCores, 2048 examples/core):

Layout: everything transposed — state h kept as hT [H=128 partitions, examples
on free dim], so the GRU recurrence is closed under the layout (no per-step
transposes). Per time step t, per 512-example group (psum tile [128, 1536] f32):

  psum[:, 0:512]    = U_z.T @ hT + M_z.T @ xghT + 1s.T @ notm_t   (z preact)
  psum[:, 512:1024] = U_r.T @ hT + M_r.T @ xghT                   (r preact)
  psum[:, 1024:1536]= U_h.T @ hT                                  (rec_h)
  z|r  = sigmoid(psum[:, 0:1024])                 (one ACT call, reads PSUM)
  t1   = (rec_h + b1_h) * r                       (fused DVE scalar_tensor_tensor)
  t2   = t1 + xgh
  hh   = tanh(t2)
  h'   = z*(h - hh) + hh                          (3 DVE tensor_tensor ops)

The only gather is xghT: dma_gather (transpose mode) from a host-precomputed
fp16 table gtab[V, 128] = emb @ W[:, 256:384] + b0_h  (256B rows).  The z/r
input projections are reconstructed algebraically instead of gathered:
  x = (xgh - b0_h) @ pinv(W_h)  (exact: xgh lies in W_h's 32-dim row space)
  xg_zr = x @ W_zr = xgh @ M + c,  M = pinv(W_h) @ W_zr  (host-precomputed)
so no second gather is needed.  dma_gather is HW-limited to ~896 idxs/call
(1024+ kills the Pool engine), so each step gathers in (896, 896, 256) splits.

Mask (token==0 freezes state): notmT[t, i] = 100 if token==0 else 0, shipped
from host; a K=1 matmul adds it to the z preactivation => z = sigmoid(.+100) = 1
exactly => h' = h.  Biases: b0_h folded into gtab; b1_h via the STT scalar;
b0/b1_zr (+ the -b0_h@M correction) via K=1 matmuls only when nonzero.

Head: dT = swish(W1.T @ hT + b1); logits per 128-example tile with examples on
partitions (lhsT = dT slice); softmax along free dim (C=3).
"""

import numpy as np
from contextlib import ExitStack

import concourse.mybir as mybir
import concourse.tile as tile
from concourse import bacc
from concourse.bass_utils import run_bass_kernel_spmd

B, T, E, H, V, D, C = 16384, 128, 32, 128, 30001, 128, 3
NCORES = 8
BC = B // NCORES
BIGM = 100.0
NIDX = 896             # max idxs per dma_gather call (HW-probed ucode limit:
                       # 896 works, 1024+ crashes the Pool engine)
USE_SPLIT = True       # split zr/g psum tiles vs one 3-bank tile
G_BUFS = 6
H_BUFS = 4
Z_BUFS = 4
TMP_BUFS = 3
NM_CH = 2
PW = 512
SIG_SPLIT = True
TANH_MERGE = True
HEAD_SHARE_PS = False
R_FIRST = True
SKIP_GATHER = False
GSPLIT_OVERRIDE = None
SGW = 1024             # psum group width (cols per z/r/g psum tile)
PS_BUFS = 1            # psum ring depth for pz/pr/pg pools

F16 = mybir.dt.float16
F32 = mybir.dt.float32
I16 = mybir.dt.int16
AF = mybir.ActivationFunctionType
OP = mybir.AluOpType
AX = mybir.AxisListType


def build_nc(bc=BC, nt=T, with_czr=False):
    """Build + compile the per-core Bass program. bc = examples per core."""
    assert bc % 512 == 0
    ng = bc // 512            # 512-example groups per step
    pw = min(PW, bc)          # width of the wide DVE ops
    npairs = bc // pw
    gperp = pw // 512
    gsplit = []
    off = 0
    while off < bc:
        n = min(NIDX, bc - off)
        gsplit.append((off, n))
        off += n
    if GSPLIT_OVERRIDE:
        gsplit = GSPLIT_OVERRIDE

    nc = bacc.Bacc("TRN2", target_bir_lowering=False, debug=False,
                   num_swdge_queues=4)
    gtab = nc.dram_tensor("gtab", [V, 128], F16, kind="ExternalInput").ap()
    idxw = nc.dram_tensor("idxw", [128, nt * bc // 16], I16, kind="ExternalInput").ap()
    uzrh = nc.dram_tensor("uzrh", [128, 384], F16, kind="ExternalInput").ap()
    mzr = nc.dram_tensor("mzr", [128, 256], F16, kind="ExternalInput").ap()
    b1h = nc.dram_tensor("b1h", [128, 1], F32, kind="ExternalInput").ap()
    w1 = nc.dram_tensor("w1", [128, 128], F16, kind="ExternalInput").ap()
    b1c = nc.dram_tensor("b1c", [128, 1], F32, kind="ExternalInput").ap()
    wout = nc.dram_tensor("wout", [128, C], F16, kind="ExternalInput").ap()
    boutw = nc.dram_tensor("boutw", [1, C], F16, kind="ExternalInput").ap()
    if with_czr:
        czr = nc.dram_tensor("czr", [1, 256], F16, kind="ExternalInput").ap()
    outp = nc.dram_tensor("outp", [128, (bc // 128) * C], F32, kind="ExternalOutput").ap()

    with tile.TileContext(nc) as tc, ExitStack() as ctx:
        wp = ctx.enter_context(tc.tile_pool(name="w", bufs=1))
        ip = ctx.enter_context(tc.tile_pool(name="idx", bufs=1))
        gp = ctx.enter_context(tc.tile_pool(name="g", bufs=G_BUFS))
        hp = ctx.enter_context(tc.tile_pool(name="h", bufs=H_BUFS))
        zp = ctx.enter_context(tc.tile_pool(name="zr", bufs=Z_BUFS))
        tp = ctx.enter_context(tc.tile_pool(name="tmp", bufs=TMP_BUFS))
        ppz = ctx.enter_context(tc.tile_pool(name="ppz", bufs=PS_BUFS, space="PSUM"))
        pprg = ctx.enter_context(tc.tile_pool(name="pprg", bufs=PS_BUFS, space="PSUM"))
        ppg = ctx.enter_context(tc.tile_pool(name="ppg", bufs=PS_BUFS, space="PSUM"))
        ph = ctx.enter_context(tc.tile_pool(name="ph", bufs=2, space="PSUM"))
        hd = ctx.enter_context(tc.tile_pool(name="hd", bufs=2))

        u_sb = wp.tile([128, 384], F16, tag="u")
        nc.sync.dma_start(u_sb[:], uzrh)
        m_sb = wp.tile([128, 256], F16, tag="mzr")
        nc.sync.dma_start(m_sb[:], mzr)
        b1h_sb = wp.tile([128, 1], F32, tag="b1h")
        nc.sync.dma_start(b1h_sb[:], b1h)
        w1_sb = wp.tile([128, 128], F16, tag="w1")
        nc.sync.dma_start(w1_sb[:], w1)
        b1c_sb = wp.tile([128, 1], F32, tag="b1c")
        nc.sync.dma_start(b1c_sb[:], b1c)
        wout_sb = wp.tile([128, C], F16, tag="wo")
        nc.sync.dma_start(wout_sb[:], wout)
        bout_sb = wp.tile([1, C], F16, tag="bo")
        nc.sync.dma_start(bout_sb[:], boutw)
        ones_sb = wp.tile([1, 128], F16, tag="ones")
        nc.vector.memset(ones_sb[:], 1.0)
        # Pin the ACT table set that contains BOTH Sigmoid and Tanh so the
        # auto-placement pass doesn't ping-pong table loads every step
        # (~1.3us per load on the ACT critical path).
        from concourse.hw_specs import get_activation_tables
        _tabs = get_activation_tables(nc.m.arch)
        _setid = next(i for i, (nm2, fs) in enumerate(_tabs.items())
                      if AF.Sigmoid in fs and AF.Tanh in fs)
        nc.scalar.add_instruction(mybir.InstLoadActFuncSet(
            name=nc.get_next_instruction_name(), ins=[], outs=[],
            act_func_set_id=_setid))
        if with_czr:
            czr_sb = wp.tile([1, 256], F16, tag="czr")
            nc.sync.dma_start(czr_sb[:], czr)
            onesbc_sb = wp.tile([1, bc], F16, tag="onesbc")
            nc.vector.memset(onesbc_sb[:], 1.0)
        idx_sb = ip.tile([128, nt * bc // 16], I16, tag="idx")
        nc.sync.dma_start(idx_sb[:], idxw)

        h = hp.tile([128, bc], F16, tag="h")
        nc.vector.memset(h[:], 0.0)

        _gq = [0]              # global gather-call counter for queue RR
        SG = min(SGW, bc)      # psum group width
        nsg = bc // SG
        BW = min(1024, bc)     # blend width (pairs groups when SG=512)
        gpb = BW // SG         # groups per blend

        def do_gather(t):
            g = gp.tile([128, 1, bc], F16, tag="g")
            for off, n in gsplit:
                nc.gpsimd.dma_gather(
                    g[:, :, off:off + n], gtab,
                    idx_sb[:, (t * bc + off) // 16:(t * bc + off + n) // 16],
                    n, n, 128, transpose=True,
                    queue_num=_gq[0] % 4,
                )
                _gq[0] += 1
            return g

        for t in range(nt):
            g = do_gather(t)
            xgh = g[:, 0, :]
            # zr layout: z in [0:bc], r in [bc:2bc] (contiguous halves so the
            # wide blend reads stay in DVE 2x mode)
            zrt = zp.tile([128, 2 * bc], F16, tag="zr")
            t1 = tp.tile([128, bc], F16, tag="t1")
            t2 = tp.tile([128, bc], F16, tag="t2")
            hh = tp.tile([128, bc], F16, tag="hh")
            dd = tp.tile([128, bc], F16, tag="dd")
            m1 = tp.tile([128, bc], F16, tag="m1")
            hnew = hp.tile([128, bc], F16, tag="h")

            pzs = [None] * nsg
            prs = [None] * nsg
            pgs = [None] * nsg
            nh = SG // 512           # matmul output <= 1 psum bank (512 f32)

            def emit_mms(sg):
                exs_base = sg * SG
                pz = ppz.tile([128, SG], F32, tag="pz")
                pr = pprg.tile([128, SG], F32, tag="prg")
                pG = ppg.tile([128, SG], F32, tag="pg")
                pzs[sg], prs[sg], pgs[sg] = pz, pr, pG

                def mm(ps_t, w, rhs, start, stop):
                    for q in range(nh):
                        qs = slice(q * 512, (q + 1) * 512)
                        rs = slice(exs_base + q * 512, exs_base + (q + 1) * 512)
                        nc.tensor.matmul(ps_t[:, qs], w, rhs[:, rs],
                                         start=start, stop=stop)

                # gather-only prefills (r first: its psum frees earlier)
                mm(pr, m_sb[:, 128:256], xgh, True, False)
                mm(pz, m_sb[:, 0:128], xgh, True, False)
                if with_czr:
                    mm(pr, czr_sb[:, 128:256], onesbc_sb, False, False)
                    mm(pz, czr_sb[:, 0:128], onesbc_sb, False, False)
                # h-dependent accumulations
                mm(pr, u_sb[:, 128:256], h, False, True)
                mm(pG, u_sb[:, 256:384], h, True, True)
                mm(pz, u_sb[:, 0:128], h, False, True)

            def emit_sig(sg):
                zsl = slice(sg * SG, (sg + 1) * SG)
                rsl = slice(bc + sg * SG, bc + (sg + 1) * SG)
                nc.scalar.activation(zrt[:, rsl], prs[sg][:], AF.Sigmoid)
                nc.scalar.activation(zrt[:, zsl], pzs[sg][:], AF.Sigmoid)

            def emit_stt(sg):
                exs = slice(sg * SG, (sg + 1) * SG)
                rsl = slice(bc + sg * SG, bc + (sg + 1) * SG)
                nc.vector.scalar_tensor_tensor(
                    t1[:, exs], pgs[sg][:], b1h_sb[:], zrt[:, rsl],
                    OP.add, OP.mult)

            def emit_blend(pi):
                exs = slice(pi * BW, (pi + 1) * BW)
                # blend: h' = z*(h - hh) + hh
                nc.vector.tensor_add(t2[:, exs], t1[:, exs], xgh[:, exs])
                nc.scalar.activation(hh[:, exs], t2[:, exs], AF.Tanh)
                nc.vector.tensor_sub(dd[:, exs], h[:, exs], hh[:, exs])
                nc.vector.tensor_mul(m1[:, exs], zrt[:, exs], dd[:, exs])
                nc.vector.tensor_add(hnew[:, exs], m1[:, exs], hh[:, exs])

            for sg in range(nsg):
                emit_mms(sg)
                emit_sig(sg)
                emit_stt(sg)
                if sg % gpb == gpb - 1:
                    emit_blend(sg // gpb)
            h = hnew

        out_sb = hd.tile([128, (bc // 128) * C], F32, tag="out")
        et_all = hd.tile([128, (bc // 128) * C], F32, tag="eta")
        ss_all = hd.tile([128, (bc // 128)], F32, tag="ssa")
        for hg in range(bc // 512):
            psd_t = ph.tile([128, 512], F32, tag="hps")
            psd = psd_t[:]
            nc.tensor.matmul(psd, w1_sb[:], h[:, hg * 512:(hg + 1) * 512], start=True, stop=True)
            sg = hd.tile([128, 512], F16, tag="sg")
            nc.scalar.activation(sg[:], psd, AF.Sigmoid, bias=b1c_sb[:])
            dt = hd.tile([128, 512], F16, tag="dt")
            # swish(d) = d * sigmoid(d), d = psd + b1
            nc.vector.scalar_tensor_tensor(dt[:], psd, b1c_sb[:], sg[:], OP.add, OP.mult)
            for sub in range(4):
                psl_t = ph.tile([128, C], F32, tag="hps")
                psl = psl_t[:]
                nc.tensor.matmul(psl, dt[:, sub * 128:(sub + 1) * 128], wout_sb[:], start=True, stop=False)
                nc.tensor.matmul(psl, ones_sb[:], bout_sb[:], start=False, stop=True)
                i = hg * 4 + sub
                nc.scalar.activation(et_all[:, i * C:(i + 1) * C], psl, AF.Exp,
                                     accum_out=ss_all[:, i:i + 1])
        rc_all = hd.tile([128, (bc // 128)], F32, tag="rc")
        nc.vector.reciprocal(rc_all[:], ss_all[:])
        for i in range(bc // 128):
            nc.vector.tensor_scalar_mul(out_sb[:, i * C:(i + 1) * C],
                                        et_all[:, i * C:(i + 1) * C], rc_all[:, i:i + 1])
        nc.sync.dma_start(outp, out_sb[:])

    nc.compile()
    return nc


def prep_tables(emb, W, U, b, W1, b1, Wout, bout):
    """Host-side weight preprocessing -> (shared input dict, with_czr flag).

    Mask trick: real-token rows of gtab lie in L = span(Wh rows, b0_h)
    (dim<=33).  Pick u ⊥ L; add alpha*u to token-0's row and u⊗[BIGM/alpha
    (z-slots), 0 (r-slots)] to M.  For real tokens u.T@xgh = 0 so nothing
    changes; for token 0 the z preact gets +BIGM => z = 1 => h frozen.
    """
    f16 = np.float16
    emb = np.asarray(emb, np.float64)
    W = np.asarray(W, np.float64)
    b = np.asarray(b, np.float64)
    Wh = W[:, 256:384]
    gtab64 = emb @ Wh + b[0, 256:384]                       # [V, 128]
    Minv = np.linalg.pinv(Wh)                               # [128, 32]
    M = Minv @ W[:, 0:256]                                  # [128, 256]
    # u orthogonal to L = span(Wh.T cols, b0_h)
    Lb = np.concatenate([Wh.T, b[0, 256:384].reshape(128, 1)], axis=1)
    Q, _ = np.linalg.qr(Lb)                                 # [128, 33]
    rng = np.random.default_rng(12345)
    gvec = rng.standard_normal(128)
    u = gvec - Q @ (Q.T @ gvec)
    u /= np.linalg.norm(u)
    ALPHA = 16.0
    gtab64[0] += ALPHA * u
    madd = np.concatenate([np.full(128, BIGM / ALPHA), np.zeros(128)])
    M = M + np.outer(u, madd)
    gtab = gtab64.astype(f16)
    M = M.astype(f16)
    # xg_zr = (xgh - b0_h) @ M + b0_zr  (+ b1_zr folded with it)
    c = (-b[0, 256:384] @ Minv @ W[:, 0:256] + b[0, 0:256] + b[1, 0:256])
    with_czr = bool(np.any(np.abs(c) > 1e-12))
    shared = {
        "gtab": gtab,
        "uzrh": np.asarray(U, np.float32).astype(f16),
        "mzr": M,  # already f16
        "b1h": np.asarray(b[1, 256:384], np.float32).reshape(128, 1).copy(),
        "w1": np.asarray(W1, np.float32).astype(f16),
        "b1c": np.asarray(b1, np.float32).reshape(128, 1).copy(),
        "wout": np.asarray(Wout, np.float32).astype(f16),
        "boutw": np.asarray(bout, np.float32).reshape(1, C).astype(f16),
    }
    if with_czr:
        shared["czr"] = c.reshape(1, 256).astype(f16)
    return shared, with_czr


def prep_idx(tokens_core, nt):
    """tokens_core [bc, nt] int -> wrapped idx tensor [128, nt*bc/16] int16."""
    bc = tokens_core.shape[0]
    tk = np.ascontiguousarray(tokens_core.astype(np.int16))
    w = tk.T.reshape(nt, bc // 16, 16).transpose(0, 2, 1)   # [t, r, c16]
    w = np.tile(w, (1, 8, 1))
    return np.ascontiguousarray(w.transpose(1, 0, 2).reshape(128, nt * bc // 16))


def assemble_out(res_core, bc=BC):
    """[128, (bc/128)*3] f32 device output -> [bc, 3]."""
    return np.ascontiguousarray(
        res_core.reshape(128, bc // 128, C).transpose(1, 0, 2).reshape(bc, C)
    )


_NC_CACHE = {}


def kernel(tokens, emb, W, U, b, W1, b1, Wout, bout):
    tokens = np.asarray(tokens)
    shared, with_czr = prep_tables(emb, W, U, b, W1, b1, Wout, bout)
    key = (BC, T, with_czr)
    if key not in _NC_CACHE:
        _NC_CACHE[key] = build_nc(BC, T, with_czr)
    nc = _NC_CACHE[key]
    in_maps = []
    for c in range(NCORES):
        m = dict(shared)
        tc = tokens[c * BC:(c + 1) * BC]
        m["idxw"] = prep_idx(tc, T)
        in_maps.append(m)
    res = run_bass_kernel_spmd(nc, in_maps, core_ids=list(range(NCORES)))
    out = np.concatenate([assemble_out(res.results[c]["outp"], BC) for c in range(NCORES)], axis=0)
    return out.astype(np.float32)

